# revision 15
# baseline (speedup 1.0000x reference)
"""Cross-attention (1x1-conv q/k/v + softmax(Q^T K) + V@attn^T) on Trainium2.

Data-parallel over batch: 8 batches -> 8 NeuronCores, one full [N,N]
attention per core; the small CxC projection weights are replicated.

Per-core device program (all matmuls, zero transposes). The two score
projections are folded into one on the host: scores = (Wq x1)^T (Wk x2)
= x1^T G x2 with G = Wk^T Wq [CxC], so x1 feeds the score matmuls raw:
  A[c,m]   = G.T @ x2              (fp16 result, c on partitions)
  vT[m,c'] = x2.T @ WvT            (bf16 result; appended ones column c'=C)
  sT[m,n]  = A.T @ x1              (fp16 operands, fp32 PSUM scores,
                                    transposed layout)
  pT[m,n]  = exp(sT - SHIFT)       (ScalarE, bf16 out; SHIFT makes per-row max
                                    subtraction unnecessary: softmax is
                                    shift-invariant and scores stay in
                                    [-150, ~110] => exp in fp32/bf16 range)
  o'[n,c'] = pT.T @ vT             (bf16; ones column accumulates row sums)
  outT[n,c] = o'[n,:C] * (1/o'[n,C])

dtype choices: everything 16-bit at 1 cycle/row on the PE. The score path
is fp16 (not bf16): input rounding is amplified sqrt(C)x through the
projections and again in the 256-length score dot products, and scores
(std ~16, range ~[-150,110]) sit in an exponent, so the 10-bit fp16
mantissa is needed -- measured end-to-end error 7.7e-3 absmax-relative vs
4.4e-3 for the old all-fp32r design (bf16 would blow the 2e-2 budget).
16-bit operands matter for speed twice over: LDWEIGHTS for a 2-byte
[128,128] stationary takes ~97ns vs 187ns for fp32r (the fp32r load
stream throttled the score phase to 227ns/matmul vs the 213ns roofline;
fp16 loads hide completely), and fp16 inputs halve the input DMA bytes
(the prologue is DMA-paced). The value path (pT, vT) is bf16: pT needs
bf16's exponent range (unnormalized exp up to e^50), and the out-phase
matmuls reload a [128,128] stationary every 257-row matmul, so their
~97ns FWL load roughly hides under the 107ns matmul. The output DMA is
fp16 (absmax ~5.6, error ~3e-3 of absmax; host upcasts).

Prologue (measured on HW): the runtime executes the first program
instruction at ~6.7us; each dma_start costs ~0.7us of serial DIRECT2D
descriptor processing on its trigger engine, and early transfers run at
~170GB/s. Both SP (sync) and Activation (scalar) have HW-DGE queues, so
the first x transfers are triggered on scalar while sync ships both
projection weights as ONE packed [P,2,KC,C] tensor (two chained weight
DMAs left the v-projection gated on wv's late arrival). x1/x2 arrive as
one DRAM tensor per transfer, host-pre-arranged into tile layout
[P, kc, n-chunk] so every partition is a single contiguous run
(descriptor-bound head transfers sped up ~1.7x). Within an engine,
transfers are priority-chained via explicit deps: the SDMA engines
round-robin across queued transfers, so an unordered queue finishes
everything at once and the PE idles (measured: two concurrent chains
just split bandwidth and delay the gating transfer). First matmul lands
~10.5-11.5us; the PE then runs gap-free to the end (~0.2us total gaps).

The PE p-state governor runs matmuls at ~1.2GHz until ~3.5us of
continuous execution, resetting on any >~1.3us idle. Warm-up dummy
matmuls were tried and removed: the gating DMA's completion jitters by
+-1us run to run, so a static dummy stream either ends early (the gap
resets the ramp) or blocks the first real matmul behind the in-order PE
queue.

Steady state (measured): score matmuls 216ns (512-row roofline 213),
out matmuls 110ns (257-row roofline 107, LDWEIGHTS-parity), zero PE
gaps; exec ~256-258us vs a ~253us structural floor (ramp + fixed
teardown + LDW parity account for the difference).

The host reassembles outT -> [B, C, H, W].

Biases are not applied: the problem spec fixes bq/bk/bv to zeros.
"""

from contextlib import ExitStack

import numpy as np

import concourse.bass as bass
import concourse.mybir as mybir
import concourse.tile as tile
from concourse import bacc, bass_utils

B, C, H, W = 8, 256, 64, 64
N = H * W          # 4096 tokens per image
P = 128            # partition count
KC = C // P        # 2 contraction chunks over channels
NMM = N // P       # 32 key-side chunks
SB = 512           # query-side superblock (score matmul free dim)
NSB = N // SB      # 8
C2 = C + 1         # value width + ones column (bf16 matmuls allow odd free)
SHIFT = 60.0       # softmax exp shift (see module docstring)

_CACHE: dict = {}
TRACE = False       # set by test harness to capture an NTFF profile
TRACE_DIR = None    # optional fixed profile output dir


def _build_program():
    f32 = mybir.dt.float32
    f32r = mybir.dt.float32r   # moving operands: full-rate PE, ~TF32 precision
    f16 = mybir.dt.float16     # score-path stationaries: fast LDWEIGHTS
    bf16 = mybir.dt.bfloat16   # value path: exp range + fast LDWEIGHTS
    exp = mybir.ActivationFunctionType.Exp
    # bacc (not raw Bass): its compile() pass splits multi-semaphore waits,
    # which walrus codegen requires (one wait per TPB instruction).
    nc = bacc.Bacc("TRN2", target_bir_lowering=False, debug=False)

    # x1/x2 arrive pre-arranged by the host as one DRAM tensor per priority-
    # chain link, each already in tile layout [P, kc, n-chunk] so every
    # partition's data is a single contiguous run (descriptor-efficient; the
    # head transfers are descriptor-bound). The first two chunks of each are
    # 512 wide so the first projections/scores start after 0.25MB each.
    XW = (1024, 1024, 1024, 1024)
    x1_cd = [nc.dram_tensor(f"x1c{i}", [P, KC, w], f16,
                            kind="ExternalInput").ap()
             for i, w in enumerate(XW)]
    x2_cd = [nc.dram_tensor(f"x2c{i}", [P, KC, w], f16,
                            kind="ExternalInput").ap()
             for i, w in enumerate(XW)]
    # both weights in one partition-major tensor -> a single trigger and a
    # single 0.25MB transfer covers wk+wv (each partition one 2KB run)
    w2_d = nc.dram_tensor("w2", [P, 2, KC, C], f16, kind="ExternalInput").ap()
    outT_d = nc.dram_tensor("outT", [N, C], f16, kind="ExternalOutput").ap()

    def r(ap):  # DRAM-side view matching the fp32r tile dtype (bit-identical)
        return ap.bitcast(f32r)

    with tile.TileContext(nc) as tc:
        with ExitStack() as ctx:
            consts = ctx.enter_context(tc.tile_pool(name="consts", bufs=1))
            acts = ctx.enter_context(tc.tile_pool(name="acts", bufs=1))
            xpool = ctx.enter_context(tc.tile_pool(name="xpool", bufs=1))

            # ---- input DMAs first: triggers cost ~670-800ns of serial
            # descriptor processing per dma_start, and nothing else in the
            # program may delay them. The first three x chunks go on the
            # scalar (Activation) HW-DGE queue, in parallel with sync
            # triggering the weights.
            w2_sb = consts.tile([P, 2, KC, C], f16, name="w2_sb")
            wk_sb = w2_sb[:, 0]
            wv_sb = w2_sb[:, 1]
            # x chunk tables: (tile, start_n, width). First two x2/x1 chunks
            # are 512 wide so the first projections start after 0.5MB.
            x2_chunks = []
            x1_chunks = []
            for nm, tbl in (("x2", x2_chunks), ("x1", x1_chunks)):
                n0 = 0
                for i, wd in enumerate(XW):
                    tbl.append((xpool.tile([P, KC, wd], f16,
                                           name=f"{nm}_{i}"), n0, wd))
                    n0 += wd
            def chain(eng, specs, prev=None):
                for dst, src in specs:
                    dma = eng.dma_start(out=dst, in_=src)
                    if prev is not None:
                        tile.add_dep_helper(dma.ins, prev.ins,
                                            reason="dma priority chain")
                    prev = dma
                return prev

            # scalar: the two 512-wide x2 head chunks, then the x1 head
            # (chained): the first k projection starts after wk + 0.25MB.
            # sync: the small weights in parallel (wk gates the first
            # matmul), then the remaining x chunks chained behind the scalar
            # head. DMA bandwidth in this phase is a shared pool, so chains
            # are serialized by priority rather than run concurrently.
            sc_tail = chain(nc.scalar, [
                (x2_chunks[0][0], x2_cd[0]),
                (x1_chunks[0][0], x1_cd[0]),
            ])
            nc.sync.dma_start(out=w2_sb, in_=w2_d)
            chain(nc.sync, [
                (x2_chunks[1][0], x2_cd[1]),
                (x2_chunks[2][0], x2_cd[2]),
                (x2_chunks[3][0], x2_cd[3]),
                (x1_chunks[1][0], x1_cd[1]),
                (x1_chunks[2][0], x1_cd[2]),
                (x1_chunks[3][0], x1_cd[3]),
            ], prev=sc_tail)

            def xs(tbl, n0, wd):
                # slice [n0, n0+wd) out of the chunk table (never straddles)
                for t, start, width in tbl:
                    if start <= n0 and n0 + wd <= start + width:
                        return t[:, :, n0 - start:n0 - start + wd]
                raise AssertionError((n0, wd))

            def xs_kc(tbl, kc, n0, wd):
                for t, start, width in tbl:
                    if start <= n0 and n0 + wd <= start + width:
                        return t[:, kc, n0 - start:n0 - start + wd]
                raise AssertionError((n0, wd))

            nbias = consts.tile([P, 1], f32)
            nc.vector.memset(nbias, -SHIFT)



            # A (folded k-side) per-superblock tiles in fp16, vT per m-chunk:
            # fine-grained deps let scores/out matmuls start before all
            # projections finish.
            k_sb = [acts.tile([P, KC, SB], f16, name=f"k_{ns}", bufs=1)
                    for ns in range(NSB)]
            vT_sb = [acts.tile([P, C2], bf16, name=f"vT_{mm}", bufs=1)
                     for mm in range(NMM)]
            for mm in range(NMM):
                nc.vector.memset(vT_sb[mm][:, C:C2], 1.0)

            # ---- pools (ps/po PSUM rotations are shared by projections
            # and the attention loop; 6 + 2 = all 8 banks) ----
            pts = ctx.enter_context(tc.tile_pool(name="pts", bufs=24))
            ps_pool = ctx.enter_context(tc.tile_pool(name="ps", bufs=3, space="PSUM"))
            po_pool = ctx.enter_context(tc.tile_pool(name="po", bufs=2, space="PSUM"))
            outp = ctx.enter_context(tc.tile_pool(name="outp", bufs=4))
            normp = ctx.enter_context(tc.tile_pool(name="normp", bufs=4))

            def emit_kqproj(ns):
                # one [P,2,SB] psum tile per n-chunk; kc-outer so consecutive
                # matmuls alternate PSUM banks
                pq = ps_pool.tile([P, 2, SB], f32, tag="ps", name=f"pq_{ns}")
                for kc in range(KC):
                    for mo in range(KC):
                        nc.tensor.matmul(
                            pq[:, mo, :],
                            lhsT=wk_sb[:, kc, mo * P:(mo + 1) * P],
                            rhs=xs_kc(x2_chunks, kc, ns * SB, SB),
                            start=(kc == 0), stop=(kc == KC - 1))
                for mo in range(KC):
                    nc.vector.tensor_copy(out=k_sb[ns][:, mo, :],
                                          in_=pq[:, mo, :])

            def emit_vproj(mm0, count):
                # m-chunks [mm0, mm0+count) of the value projection; pairs
                # of accumulators from the po rotation alternate banks
                for pr in range(count // 2):
                    pv = [po_pool.tile([P, C], f32, tag="po",
                                       name=f"pv_{mm0}_{pr}_{i}")
                          for i in range(2)]
                    for kc in range(KC):
                        for i in range(2):
                            mm = mm0 + pr * 2 + i
                            nc.tensor.matmul(
                                pv[i],
                                lhsT=xs_kc(x2_chunks, kc, mm * P, P),
                                rhs=wv_sb[:, kc, :],
                                start=(kc == 0), stop=(kc == KC - 1))
                    for i in range(2):
                        nc.vector.tensor_copy(
                            out=vT_sb[mm0 + pr * 2 + i][:, 0:C],
                            in_=pv[i])

            def emit_scores(sb, t, pt_tiles):
                ps = ps_pool.tile([P, 2, SB], f32, tag="ps",
                                  name=f"ps_{sb}_{t}")
                for kc in range(KC):   # kc-outer: banks alternate A B A B
                    for i in range(2):
                        koff = (t * 2 + i) * P
                        kt = k_sb[koff // SB]
                        nc.tensor.matmul(
                            ps[:, i, :],
                            lhsT=kt[:, kc, koff % SB:koff % SB + P],
                            rhs=xs_kc(x1_chunks, kc, sb * SB, SB),
                            start=(kc == 0), stop=(kc == KC - 1))
                pt = pts.tile([P, 2, SB], bf16, tag="pt")
                nc.scalar.activation(out=pt, in_=ps, func=exp,
                                     bias=nbias, scale=1.0)
                pt_tiles.append(pt)

            def emit_out(sb, pt_tiles):
                # j-outer: one live out-accumulator bank at a time.
                for j in range(SB // P):
                    po = po_pool.tile([P, C2], f32, tag="po",
                                      name=f"po_{sb}_{j}")
                    for mm in range(NMM):
                        nc.tensor.matmul(
                            po,
                            lhsT=pt_tiles[mm // 2][:, mm % 2,
                                                   j * P:(j + 1) * P],
                            rhs=vT_sb[mm],
                            start=(mm == 0), stop=(mm == NMM - 1))
                    rc = normp.tile([P, 1], f32, tag="rc")
                    nc.vector.reciprocal(rc, po[:, C:C + 1])
                    ot = outp.tile([P, C], f16, tag="ot")
                    nc.vector.tensor_scalar_mul(ot, po[:, 0:C], rc)
                    n0 = sb * SB + j * P
                    nc.sync.dma_start(out=outT_d[n0:n0 + P, :], in_=ot)

            # ---- prologue: k/v projections hand-interleaved with the first
            # superblock's scores, following the DMA arrival order, so the PE
            # never drains while x2/x1 chunks trickle in ----
            pt0 = []
            for qt in range(4):
                emit_kqproj(qt * 2)
                emit_kqproj(qt * 2 + 1)
                emit_vproj(qt * 8, 8)
                for t in range(qt * 4, qt * 4 + 4):
                    emit_scores(0, t, pt0)
            emit_out(0, pt0)

            for sb in range(1, NSB):
                pt_tiles = []
                for t in range(NMM // 2):
                    emit_scores(sb, t, pt_tiles)
                emit_out(sb, pt_tiles)
    nc.compile()
    return nc


def _get_program():
    if "nc" not in _CACHE:
        _CACHE["nc"] = _build_program()
    return _CACHE["nc"]


def kernel(**inputs) -> np.ndarray:
    # per-chunk tile layout [partition, kc, n-chunk] with channel c=kc*128+p
    XW = (1024, 1024, 1024, 1024)
    def arrange(x):
        x = np.asarray(x, np.float16).reshape(B, KC, P, N).transpose(0, 2, 1, 3)
        chunks, n0 = [], 0
        for w in XW:
            chunks.append(np.ascontiguousarray(x[:, :, :, n0:n0 + w]))
            n0 += w
        return chunks
    x1c = arrange(inputs["x1"])
    x2c = arrange(inputs["x2"])
    # scores = (Wq x1)^T (Wk x2) = x1^T (Wq^T Wk) x2: fold both score
    # projections into one by shipping G = Wk^T Wq as the k-side weight;
    # x1 then feeds the score matmuls raw (saves 32 matmuls/core and one
    # fp32r rounding on the q side).
    G = (np.asarray(inputs["Wk"], np.float64).T
         @ np.asarray(inputs["Wq"], np.float64))
    wkT = G.astype(np.float16)
    wvT = np.asarray(inputs["Wv"], np.float16).T
    # [P, 2, KC, C] partition-major pack of (G, WvT); channel c = kc*128 + p
    w2 = np.ascontiguousarray(
        np.stack([wkT.reshape(KC, P, C), wvT.reshape(KC, P, C)],
                 axis=0).transpose(2, 0, 1, 3))

    in_maps = []
    for b in range(B):
        m = {"w2": w2}
        for i in range(len(XW)):
            m[f"x1c{i}"] = x1c[i][b]
            m[f"x2c{i}"] = x2c[i][b]
        in_maps.append(m)
    nc = _get_program()
    res = bass_utils.run_bass_kernel_spmd(nc, in_maps, core_ids=list(range(B)),
                                          trace=TRACE, tmpdir=TRACE_DIR)
    _CACHE["last_results"] = res
    out = np.empty((B, C, N), np.float32)
    for b in range(B):
        out[b] = res.results[b]["outT"].T.astype(np.float32)
    return out.reshape(B, C, H, W)


if __name__ == "__main__":
    nc = _build_program()
    n = sum(len(b.instructions) for b in nc.m.functions[0].blocks)
    print(f"program built ok: {n} instructions")


# revision 17
# speedup vs baseline: 1.0000x; 1.0000x over previous
"""Cross-attention (1x1-conv q/k/v + softmax(Q^T K) + V@attn^T) on Trainium2.

Data-parallel over batch: 8 batches -> 8 NeuronCores, one full [N,N]
attention per core; the small CxC projection weights are replicated.

Per-core device program (all matmuls, zero transposes). The two score
projections are folded into one on the host: scores = (Wq x1)^T (Wk x2)
= x1^T G x2 with G = Wk^T Wq [CxC], so x1 feeds the score matmuls raw:
  A[c,m]   = G.T @ x2              (fp16 result, c on partitions)
  vT[m,c'] = x2.T @ WvT            (bf16 result; appended ones column c'=C)
  sT[m,n]  = A.T @ x1              (fp16 operands, fp32 PSUM scores,
                                    transposed layout)
  pT[m,n]  = exp(sT - SHIFT)       (ScalarE, bf16 out; SHIFT makes per-row max
                                    subtraction unnecessary: softmax is
                                    shift-invariant and scores stay in
                                    [-150, ~110] => exp in fp32/bf16 range)
  o'[n,c'] = pT.T @ vT             (bf16; ones column accumulates row sums)
  outT[n,c] = o'[n,:C] * (1/o'[n,C])

dtype choices: everything 16-bit at 1 cycle/row on the PE. The score path
is fp16 (not bf16): input rounding is amplified sqrt(C)x through the
projections and again in the 256-length score dot products, and scores
(std ~16, range ~[-150,110]) sit in an exponent, so the 10-bit fp16
mantissa is needed -- measured end-to-end error 7.7e-3 absmax-relative vs
4.4e-3 for the old all-fp32r design (bf16 would blow the 2e-2 budget).
16-bit operands matter for speed twice over: LDWEIGHTS for a 2-byte
[128,128] stationary takes ~97ns vs 187ns for fp32r (the fp32r load
stream throttled the score phase to 227ns/matmul vs the 213ns roofline;
fp16 loads hide completely), and fp16 inputs halve the input DMA bytes
(the prologue is DMA-paced). The value path (pT, vT) is bf16: pT needs
bf16's exponent range (unnormalized exp up to e^50), and the out-phase
matmuls reload a [128,128] stationary every 257-row matmul, so their
~97ns FWL load roughly hides under the 107ns matmul. The output DMA is
fp16 (absmax ~5.6, error ~3e-3 of absmax; host upcasts).

Prologue (measured on HW): the runtime executes the first program
instruction at ~6.7us; each dma_start costs ~0.7us of serial DIRECT2D
descriptor processing on its trigger engine, and early transfers run at
~170GB/s. Both SP (sync) and Activation (scalar) have HW-DGE queues, so
the first x transfers are triggered on scalar while sync ships both
projection weights as ONE packed [P,2,KC,C] tensor (two chained weight
DMAs left the v-projection gated on wv's late arrival). x1/x2 arrive as
one DRAM tensor per transfer, host-pre-arranged into tile layout
[P, kc, n-chunk] so every partition is a single contiguous run
(descriptor-bound head transfers sped up ~1.7x). Within an engine,
transfers are priority-chained via explicit deps: the SDMA engines
round-robin across queued transfers, so an unordered queue finishes
everything at once and the PE idles (measured: two concurrent chains
just split bandwidth and delay the gating transfer). First matmul lands
~10.5-11.5us; the PE then runs gap-free to the end (~0.2us total gaps).

The PE p-state governor runs matmuls at ~1.2GHz until ~3.5us of
continuous execution, resetting on any >~1.3us idle. Warm-up dummy
matmuls were tried and removed: the gating DMA's completion jitters by
+-1us run to run, so a static dummy stream either ends early (the gap
resets the ramp) or blocks the first real matmul behind the in-order PE
queue.

Steady state (measured): score matmuls 216ns (512-row roofline 213),
out matmuls 110ns (257-row roofline 107, LDWEIGHTS-parity), zero PE
gaps; exec ~256-258us vs a ~253us structural floor (ramp + fixed
teardown + LDW parity account for the difference).

The host reassembles outT -> [B, C, H, W].

Biases are not applied: the problem spec fixes bq/bk/bv to zeros.
"""

from contextlib import ExitStack

import numpy as np

import concourse.bass as bass
import concourse.mybir as mybir
import concourse.tile as tile
from concourse import bacc, bass_utils

B, C, H, W = 8, 256, 64, 64
N = H * W          # 4096 tokens per image
P = 128            # partition count
KC = C // P        # 2 contraction chunks over channels
NMM = N // P       # 32 key-side chunks
SB = 512           # query-side superblock (score matmul free dim)
NSB = N // SB      # 8
C2 = C + 1         # value width + ones column (bf16 matmuls allow odd free)
SHIFT = 60.0       # softmax exp shift (see module docstring)

_CACHE: dict = {}
TRACE = False       # set by test harness to capture an NTFF profile
TRACE_DIR = None    # optional fixed profile output dir


def _build_program():
    f32 = mybir.dt.float32
    f32r = mybir.dt.float32r   # moving operands: full-rate PE, ~TF32 precision
    f16 = mybir.dt.float16     # score-path stationaries: fast LDWEIGHTS
    bf16 = mybir.dt.bfloat16   # value path: exp range + fast LDWEIGHTS
    exp = mybir.ActivationFunctionType.Exp
    # bacc (not raw Bass): its compile() pass splits multi-semaphore waits,
    # which walrus codegen requires (one wait per TPB instruction).
    nc = bacc.Bacc("TRN2", target_bir_lowering=False, debug=False)

    # x1/x2 arrive pre-arranged by the host as one DRAM tensor per priority-
    # chain link (a quarter each), already in tile layout [P, kc, n-chunk] so
    # every partition's data is a single contiguous run (the head transfers
    # are descriptor-bound).
    XW = (1024, 1024, 1024, 1024)
    x1_cd = [nc.dram_tensor(f"x1c{i}", [P, KC, w], f16,
                            kind="ExternalInput").ap()
             for i, w in enumerate(XW)]
    x2_cd = [nc.dram_tensor(f"x2c{i}", [P, KC, w], f16,
                            kind="ExternalInput").ap()
             for i, w in enumerate(XW)]
    # both weights in one partition-major tensor -> a single trigger and a
    # single 0.25MB transfer covers wk+wv (each partition one 2KB run)
    w2_d = nc.dram_tensor("w2", [P, 2, KC, C], f16, kind="ExternalInput").ap()
    outT_d = nc.dram_tensor("outT", [N, C], f16, kind="ExternalOutput").ap()

    def r(ap):  # DRAM-side view matching the fp32r tile dtype (bit-identical)
        return ap.bitcast(f32r)

    with tile.TileContext(nc) as tc:
        with ExitStack() as ctx:
            consts = ctx.enter_context(tc.tile_pool(name="consts", bufs=1))
            acts = ctx.enter_context(tc.tile_pool(name="acts", bufs=1))
            xpool = ctx.enter_context(tc.tile_pool(name="xpool", bufs=1))

            # ---- input DMAs first: triggers cost ~670-800ns of serial
            # descriptor processing per dma_start, and nothing else in the
            # program may delay them. The first x chunks go on the scalar
            # (Activation) HW-DGE queue, in parallel with sync triggering
            # the weights.
            w2_sb = consts.tile([P, 2, KC, C], f16, name="w2_sb")
            wk_sb = w2_sb[:, 0]
            wv_sb = w2_sb[:, 1]
            # x chunk tables: (tile, start_n, width), one tile per DMA
            x2_chunks = []
            x1_chunks = []
            for nm, tbl in (("x2", x2_chunks), ("x1", x1_chunks)):
                n0 = 0
                for i, wd in enumerate(XW):
                    tbl.append((xpool.tile([P, KC, wd], f16,
                                           name=f"{nm}_{i}"), n0, wd))
                    n0 += wd
            def chain(eng, specs, prev=None):
                for dst, src in specs:
                    dma = eng.dma_start(out=dst, in_=src)
                    if prev is not None:
                        tile.add_dep_helper(dma.ins, prev.ins,
                                            reason="dma priority chain")
                    prev = dma
                return prev

            # scalar: the two 512-wide x2 head chunks, then the x1 head
            # (chained): the first k projection starts after wk + 0.25MB.
            # sync: the small weights in parallel (wk gates the first
            # matmul), then the remaining x chunks chained behind the scalar
            # head. DMA bandwidth in this phase is a shared pool, so chains
            # are serialized by priority rather than run concurrently.
            sc_tail = chain(nc.scalar, [
                (x2_chunks[0][0], x2_cd[0]),
                (x1_chunks[0][0], x1_cd[0]),
            ])
            nc.sync.dma_start(out=w2_sb, in_=w2_d)
            chain(nc.sync, [
                (x2_chunks[1][0], x2_cd[1]),
                (x2_chunks[2][0], x2_cd[2]),
                (x2_chunks[3][0], x2_cd[3]),
                (x1_chunks[1][0], x1_cd[1]),
                (x1_chunks[2][0], x1_cd[2]),
                (x1_chunks[3][0], x1_cd[3]),
            ], prev=sc_tail)

            def xs(tbl, n0, wd):
                # slice [n0, n0+wd) out of the chunk table (never straddles)
                for t, start, width in tbl:
                    if start <= n0 and n0 + wd <= start + width:
                        return t[:, :, n0 - start:n0 - start + wd]
                raise AssertionError((n0, wd))

            def xs_kc(tbl, kc, n0, wd):
                for t, start, width in tbl:
                    if start <= n0 and n0 + wd <= start + width:
                        return t[:, kc, n0 - start:n0 - start + wd]
                raise AssertionError((n0, wd))

            nbias = consts.tile([P, 1], f32)
            nc.vector.memset(nbias, -SHIFT)



            # A (folded k-side) per-superblock tiles in fp16, vT per m-chunk:
            # fine-grained deps let scores/out matmuls start before all
            # projections finish.
            k_sb = [acts.tile([P, KC, SB], f16, name=f"k_{ns}", bufs=1)
                    for ns in range(NSB)]
            vT_sb = [acts.tile([P, C2], bf16, name=f"vT_{mm}", bufs=1)
                     for mm in range(NMM)]
            for mm in range(NMM):
                nc.vector.memset(vT_sb[mm][:, C:C2], 1.0)

            # ---- pools (ps/po PSUM rotations are shared by projections
            # and the attention loop; 6 + 2 = all 8 banks) ----
            pts = ctx.enter_context(tc.tile_pool(name="pts", bufs=24))
            ps_pool = ctx.enter_context(tc.tile_pool(name="ps", bufs=3, space="PSUM"))
            po_pool = ctx.enter_context(tc.tile_pool(name="po", bufs=2, space="PSUM"))
            outp = ctx.enter_context(tc.tile_pool(name="outp", bufs=4))
            normp = ctx.enter_context(tc.tile_pool(name="normp", bufs=4))

            def emit_kqproj(ns):
                # one [P,2,SB] psum tile per n-chunk; kc-outer so consecutive
                # matmuls alternate PSUM banks
                pq = ps_pool.tile([P, 2, SB], f32, tag="ps", name=f"pq_{ns}")
                for kc in range(KC):
                    for mo in range(KC):
                        nc.tensor.matmul(
                            pq[:, mo, :],
                            lhsT=wk_sb[:, kc, mo * P:(mo + 1) * P],
                            rhs=xs_kc(x2_chunks, kc, ns * SB, SB),
                            start=(kc == 0), stop=(kc == KC - 1))
                for mo in range(KC):
                    nc.vector.tensor_copy(out=k_sb[ns][:, mo, :],
                                          in_=pq[:, mo, :])

            def emit_vproj(mm0, count):
                # m-chunks [mm0, mm0+count) of the value projection; pairs
                # of accumulators from the po rotation alternate banks
                for pr in range(count // 2):
                    pv = [po_pool.tile([P, C], f32, tag="po",
                                       name=f"pv_{mm0}_{pr}_{i}")
                          for i in range(2)]
                    for kc in range(KC):
                        for i in range(2):
                            mm = mm0 + pr * 2 + i
                            nc.tensor.matmul(
                                pv[i],
                                lhsT=xs_kc(x2_chunks, kc, mm * P, P),
                                rhs=wv_sb[:, kc, :],
                                start=(kc == 0), stop=(kc == KC - 1))
                    for i in range(2):
                        nc.vector.tensor_copy(
                            out=vT_sb[mm0 + pr * 2 + i][:, 0:C],
                            in_=pv[i])

            def emit_scores(sb, t, pt_tiles):
                ps = ps_pool.tile([P, 2, SB], f32, tag="ps",
                                  name=f"ps_{sb}_{t}")
                for kc in range(KC):   # kc-outer: banks alternate A B A B
                    for i in range(2):
                        koff = (t * 2 + i) * P
                        kt = k_sb[koff // SB]
                        nc.tensor.matmul(
                            ps[:, i, :],
                            lhsT=kt[:, kc, koff % SB:koff % SB + P],
                            rhs=xs_kc(x1_chunks, kc, sb * SB, SB),
                            start=(kc == 0), stop=(kc == KC - 1))
                pt = pts.tile([P, 2, SB], bf16, tag="pt")
                nc.scalar.activation(out=pt, in_=ps, func=exp,
                                     bias=nbias, scale=1.0)
                pt_tiles.append(pt)

            def emit_out(sb, pt_tiles):
                # j-outer: one live out-accumulator bank at a time.
                for j in range(SB // P):
                    po = po_pool.tile([P, C2], f32, tag="po",
                                      name=f"po_{sb}_{j}")
                    for mm in range(NMM):
                        nc.tensor.matmul(
                            po,
                            lhsT=pt_tiles[mm // 2][:, mm % 2,
                                                   j * P:(j + 1) * P],
                            rhs=vT_sb[mm],
                            start=(mm == 0), stop=(mm == NMM - 1))
                    rc = normp.tile([P, 1], f32, tag="rc")
                    nc.vector.reciprocal(rc, po[:, C:C + 1])
                    n0 = sb * SB + j * P
                    if sb == NSB - 1 and j == SB // P - 1:
                        # very last block: the post-matmul chain (recip, mul,
                        # ~0.6us serial DMA-trigger, transfer) is pure drain.
                        # Split it into c-halves with the two triggers on
                        # different HW-DGE engines so the halves overlap.
                        ha = outp.tile([P, C // 2], f16, tag="ot")
                        nc.vector.tensor_scalar_mul(ha, po[:, 0:C // 2], rc)
                        nc.sync.dma_start(
                            out=outT_d[n0:n0 + P, 0:C // 2], in_=ha)
                        hb = outp.tile([P, C // 2], f16, tag="ot")
                        nc.vector.tensor_scalar_mul(hb, po[:, C // 2:C], rc)
                        nc.scalar.dma_start(
                            out=outT_d[n0:n0 + P, C // 2:C], in_=hb)
                    else:
                        ot = outp.tile([P, C], f16, tag="ot")
                        nc.vector.tensor_scalar_mul(ot, po[:, 0:C], rc)
                        nc.sync.dma_start(out=outT_d[n0:n0 + P, :], in_=ot)

            # ---- prologue: k/v projections hand-interleaved with the first
            # superblock's scores, following the DMA arrival order, so the PE
            # never drains while x2/x1 chunks trickle in ----
            pt0 = []
            for qt in range(4):
                emit_kqproj(qt * 2)
                emit_kqproj(qt * 2 + 1)
                emit_vproj(qt * 8, 8)
                for t in range(qt * 4, qt * 4 + 4):
                    emit_scores(0, t, pt0)
            emit_out(0, pt0)

            for sb in range(1, NSB):
                pt_tiles = []
                for t in range(NMM // 2):
                    emit_scores(sb, t, pt_tiles)
                emit_out(sb, pt_tiles)
    nc.compile()
    return nc


def _get_program():
    if "nc" not in _CACHE:
        _CACHE["nc"] = _build_program()
    return _CACHE["nc"]


def kernel(**inputs) -> np.ndarray:
    # per-chunk tile layout [partition, kc, n-chunk] with channel c=kc*128+p
    XW = (1024, 1024, 1024, 1024)
    def arrange(x):
        x = np.asarray(x, np.float16).reshape(B, KC, P, N).transpose(0, 2, 1, 3)
        chunks, n0 = [], 0
        for w in XW:
            chunks.append(np.ascontiguousarray(x[:, :, :, n0:n0 + w]))
            n0 += w
        return chunks
    x1c = arrange(inputs["x1"])
    x2c = arrange(inputs["x2"])
    # scores = (Wq x1)^T (Wk x2) = x1^T (Wq^T Wk) x2: fold both score
    # projections into one by shipping G = Wk^T Wq as the k-side weight;
    # x1 then feeds the score matmuls raw (saves 32 matmuls/core and one
    # fp32r rounding on the q side).
    G = (np.asarray(inputs["Wk"], np.float64).T
         @ np.asarray(inputs["Wq"], np.float64))
    wkT = G.astype(np.float16)
    wvT = np.asarray(inputs["Wv"], np.float16).T
    # [P, 2, KC, C] partition-major pack of (G, WvT); channel c = kc*128 + p
    w2 = np.ascontiguousarray(
        np.stack([wkT.reshape(KC, P, C), wvT.reshape(KC, P, C)],
                 axis=0).transpose(2, 0, 1, 3))

    in_maps = []
    for b in range(B):
        m = {"w2": w2}
        for i in range(len(XW)):
            m[f"x1c{i}"] = x1c[i][b]
            m[f"x2c{i}"] = x2c[i][b]
        in_maps.append(m)
    nc = _get_program()
    res = bass_utils.run_bass_kernel_spmd(nc, in_maps, core_ids=list(range(B)),
                                          trace=TRACE, tmpdir=TRACE_DIR)
    _CACHE["last_results"] = res
    out = np.empty((B, C, N), np.float32)
    for b in range(B):
        out[b] = res.results[b]["outT"].T.astype(np.float32)
    return out.reshape(B, C, H, W)


if __name__ == "__main__":
    nc = _build_program()
    n = sum(len(b.instructions) for b in nc.m.functions[0].blocks)
    print(f"program built ok: {n} instructions")
